# revision 12
# baseline (speedup 1.0000x reference)
"""Longformer sliding-window self-attention on 8 Trainium2 NeuronCores.

Problem: hidden_states [2, 4096, 1024], 16 heads x 64 dim, window w=256.
  q = (X@Wq + bq)/sqrt(64); k = X@Wk + bk; v = X@Wv + bv
  Block-banded attention: query block n (256 queries) attends key blocks
  n-1, n, n+1 with band |ky - qx - w| <= w plus sequence bounds.

Sharding: head-parallel. Each of the 8 cores computes a 128-column slice of
the QKV projection output (= 2 heads) for the full batch/sequence, runs the
banded attention for its 2 heads, and writes out [2, 4096, 128]. The host
concatenates slices along the embedding axis.

Device-side layout choices (all matmuls are fp32r, 1 PE cycle/row at N>=256):
  - Host passes X^T [1024, 8192] so projection matmuls need no on-device
    transpose of X:  out = W_slice.T @ X^T = Q^T/K^T/V^T [128 cols, tokens].
  - Scores are computed transposed, S^T [key, query], per 128-key chunk:
    S^T_c = K^T_chunk.T @ Q^T  -> [128, 256].  Softmax runs along the free
    (query) axis only for exp; normalization sums over keys come from an
    extra ones-column appended to V (so the PV matmul emits both attn^T and
    the softmax denominator Z), avoiding any partition-dim reduction.
  - Band masks are added into the score PSUM with an identity-weight matmul
    before exp (exp(-1.25e8) == 0 exactly in fp32).
  - V is produced as V^T then PE-transposed into natural [key, dim] chunks
    (PV stationary operand needs [key, dim]).
  - attn^T [65, 256] PSUM is PE-transposed back to [queries, 65]; the last
    column holds Z, so a reciprocal + per-partition scale finishes softmax.

Sequence bounds: key chunks that fall outside [0, S) are simply skipped
(first/last block contract over 4 chunks instead of 6).
"""

import numpy as np

import concourse.bass as bass
import concourse.mybir as mybir
import concourse.tile as tile
from concourse.vector_clock import ScopedClock
from concourse.bass_utils import run_bass_kernel_spmd
from contextlib import ExitStack

# Problem shape (hardcoded per the harness contract).
B, S, E = 2, 4096, 1024
H, D, W = 16, 64, 256
NB = S // W          # 16 query blocks per sequence
NCORE = 8
HL = H // NCORE      # 2 heads per core
C = E // NCORE       # 128 projection output columns per core
TC = 512             # projection token-chunk (N of the projection matmuls)
NT = B * S // TC     # 16 projection chunks
KCH = E // 128       # 8 contraction chunks of the projection
SP = S + 2 * W       # padded key extent per sequence (offset +W)
NCH = SP // 128      # 36 key chunks per sequence in padded coords
MASKVAL = -1e9

f32 = mybir.dt.float32
f32r = mybir.dt.float32r
AF = mybir.ActivationFunctionType


class _TileContext(tile.TileContext):
    """TileContext whose exit drain splits semaphore waits.

    The walrus build in this container rejects >1 sync wait on one
    instruction ("Too many sync wait commands"), while Tile's exit drain
    accumulates one wait per outstanding semaphore.  Carry each wait on its
    own drain instruction instead.
    """

    MAX_WAITS = 1

    def _drain_and_barrier(self, tick_clock, wait_clock):
        drain_inst = self.nc.sync.drain()
        wait_clock.add_sem_waits(
            drain_inst.ins, ScopedClock({None: tick_clock.global_clock})
        )
        si = drain_inst.ins.sync_info
        waits = list(si.on_wait or [])
        if len(waits) > self.MAX_WAITS:
            si.on_wait = waits[: self.MAX_WAITS]
            rest = waits[self.MAX_WAITS :]
            while rest:
                d2 = self.nc.sync.drain()
                si2 = d2.ins.sync_info
                if si2 is None:
                    si2 = mybir.SyncInfo(on_wait=[], on_update=[])
                    d2.ins.sync_info = si2
                si2.on_wait = rest[: self.MAX_WAITS]
                rest = rest[self.MAX_WAITS :]
        self.nc.all_engine_barrier()
        assert self.sems is not None
        popped = self.nc._tile_sem_poison_stack.pop()
        assert popped is self._sem_poison
        self.nc.clear_and_free_semaphores(list(self.sems.allocated().values()))
        self.nc.all_engine_barrier()


def _split_sync_waits(nc, limit=1):
    """Move excess per-instruction sem waits onto same-engine NoOp carriers.

    An engine executes its instruction stream in order, so a wait hoisted
    onto a NoOp immediately before the instruction blocks the engine at the
    same program point.
    """
    n_new = 0
    for fn in nc.m.functions:
        for bb in fn.blocks:
            out = []
            for inst in bb.instructions:
                si = getattr(inst, "sync_info", None)
                waits = list(si.on_wait) if si is not None and si.on_wait else []
                if len(waits) > limit:
                    extra = waits[: len(waits) - limit]
                    si.on_wait = waits[len(waits) - limit :]
                    while extra:
                        chunk = extra[:limit]
                        extra = extra[limit:]
                        nop = mybir.InstNoOp(
                            name=f"waitsplit-{nc.next_id()}", ins=[], outs=[]
                        )
                        nop.engine = inst.engine
                        nop.sync_info = mybir.SyncInfo(on_wait=chunk, on_update=[])
                        out.append(nop)
                        n_new += 1
                out.append(inst)
            bb.instructions = out
    return n_new


def _emit_with_pools(nc, tc, aps, sing, stores):
    """Emit one full forward pass (projection + attention) into tc."""
    xt_ap = aps["xt"]
    out_ap = aps["out"]

    # Constants.
    id_f = sing.tile([128, 128], f32)
    nc.sync.dma_start(id_f, aps["idn"])
    id_r = sing.tile([128, 128], f32r)
    nc.sync.dma_start(id_r, aps["idn"].bitcast(f32r))
    msk = sing.tile([128, 4, W], f32r)
    nc.sync.dma_start(msk, aps["msk"].rearrange("m p x -> p m x").bitcast(f32r))

    w_sbs = []
    b_sbs = []
    for nm in ("q", "k", "v"):
        w_sb = sing.tile([128, KCH, C], f32r, name=f"w{nm}_sb")
        nc.sync.dma_start(
            w_sb, aps["w" + nm].rearrange("(kc p) c -> p kc c", p=128).bitcast(f32r)
        )
        b_sb = sing.tile([128, 1], f32, name=f"b{nm}_sb")
        nc.sync.dma_start(b_sb, aps["b" + nm][:, None])
        w_sbs.append(w_sb)
        b_sbs.append(b_sb)

    # Persistent per-core stores: Q^T, K^T (padded key coords), V chunks.
    QT = stores.tile([128, B * S], f32r)
    KT = stores.tile([128, B * SP], f32r)
    VS = stores.tile([128, B, HL, NCH, D + 1], f32r)
    # ones-column: softmax denominator accumulates through the PV matmul.
    # (memset can't write f32r in this walrus build; broadcast-DMA instead.)
    ones_bcast = bass.AP(
        tensor=aps["ones"].tensor,
        offset=0,
        ap=[[0, 128], [0, NCH - 4]],
    ).bitcast(f32r)
    for b in range(B):
        for h in range(HL):
            nc.sync.dma_start(VS[:, b, h, 2 : NCH - 2, D], ones_bcast)

    # ---- Phase 1: QKV projections (transposed outputs) ----
    with ExitStack() as p1:
        xpool = p1.enter_context(tc.tile_pool(name="xpool", bufs=2))
        vtp = p1.enter_context(tc.tile_pool(name="vtp", bufs=2))
        pps = p1.enter_context(tc.tile_pool(name="pps", bufs=2, space="PSUM"))

        xt_re = xt_ap.rearrange("(kc p) n -> p kc n", p=128)
        for t in range(NT):
            b_t, sub_t = divmod(t, S // TC)
            toff = sub_t * TC
            xt_t = xpool.tile([128, KCH, TC], f32r, tag="xt")
            nc.sync.dma_start(
                xt_t, xt_re[:, :, t * TC : (t + 1) * TC].bitcast(f32r)
            )
            for ip, nm in enumerate("qkv"):
                ps = pps.tile([128, TC], f32, tag=f"ps{nm}", name=f"ps{nm}")
                for kc in range(KCH):
                    nc.tensor.matmul(
                        ps,
                        w_sbs[ip][:, kc, :],
                        xt_t[:, kc, :],
                        start=(kc == 0),
                        stop=(kc == KCH - 1),
                    )
                if nm == "q":
                    nc.scalar.activation(
                        QT[:, t * TC : (t + 1) * TC], ps, AF.Identity, bias=b_sbs[0]
                    )
                elif nm == "k":
                    off = b_t * SP + W + toff
                    nc.scalar.activation(
                        KT[:, off : off + TC], ps, AF.Identity, bias=b_sbs[1]
                    )
                else:
                    vt = vtp.tile([128, TC], f32, tag="vt", name="vt")
                    nc.scalar.activation(vt, ps, AF.Identity, bias=b_sbs[2])
                    for h in range(HL):
                        for sub in range(TC // 128):
                            pvt = pps.tile([128, D], f32, tag="pvt", name="pvt")
                            # identity slice picked at the same base partition
                            # as the input (matmul requires matching bases)
                            nc.tensor.transpose(
                                pvt,
                                vt[h * D : (h + 1) * D, sub * 128 : (sub + 1) * 128],
                                id_f[h * D : (h + 1) * D, h * D : (h + 1) * D],
                            )
                            ch = (W + toff) // 128 + sub
                            nc.scalar.activation(
                                VS[:, b_t, h, ch, 0:D], pvt, AF.Copy
                            )

    # ---- Phase 2: banded attention ----
    with ExitStack() as p2:
        spool = p2.enter_context(tc.tile_pool(name="spool", bufs=2))
        fpool = p2.enter_context(tc.tile_pool(name="fpool", bufs=3))
        ps2 = p2.enter_context(tc.tile_pool(name="ps2", bufs=3, space="PSUM"))
        pa = p2.enter_context(tc.tile_pool(name="pa", bufs=2, space="PSUM"))
        pt = p2.enter_context(tc.tile_pool(name="pt", bufs=2, space="PSUM"))

        MI = {0: 0, 1: 1, 4: 2, 5: 3}  # chunk -> band-mask variant
        for b in range(B):
            for h in range(HL):
                for n in range(NB):
                    c_lo = 2 if n == 0 else 0
                    c_hi = 4 if n == NB - 1 else 6
                    exps = {}
                    for c in range(c_lo, c_hi):
                        sps = ps2.tile([128, W], f32, tag="sps", name="sps")
                        first = True
                        if c in MI:
                            nc.tensor.matmul(
                                sps, id_r, msk[:, MI[c], :], start=True, stop=False
                            )
                            first = False
                        koff = b * SP + n * W + c * 128
                        nc.tensor.matmul(
                            sps,
                            KT[h * D : (h + 1) * D, koff : koff + 128],
                            QT[h * D : (h + 1) * D, b * S + n * W : b * S + (n + 1) * W],
                            start=first,
                            stop=True,
                        )
                        ex = spool.tile([128, W], f32r, tag=f"ex{c}", name=f"ex{c}")
                        nc.scalar.activation(ex, sps, AF.Exp, scale=1.0 / np.sqrt(D))
                        exps[c] = ex
                    aps_t = pa.tile([D + 1, W], f32, tag="aps", name="aps")
                    for i, c in enumerate(range(c_lo, c_hi)):
                        nc.tensor.matmul(
                            aps_t,
                            VS[:, b, h, 2 * n + c, :],
                            exps[c],
                            start=(i == 0),
                            stop=(c == c_hi - 1),
                        )
                    patt = fpool.tile([D + 1, W], f32, tag="patt", name="patt")
                    nc.vector.tensor_copy(patt, aps_t)
                    for half in range(2):
                        tp = pt.tile([128, D + 1], f32, tag="tp", name="tp")
                        nc.tensor.transpose(
                            tp,
                            patt[:, half * 128 : (half + 1) * 128],
                            id_f[0 : D + 1, 0 : D + 1],
                        )
                        rc = fpool.tile([128, 1], f32, tag="rc", name="rc")
                        nc.vector.reciprocal(rc, tp[:, D : D + 1])
                        ao = fpool.tile([128, D], f32, tag="ao", name="ao")
                        nc.vector.tensor_scalar_mul(ao, tp[:, 0:D], rc)
                        r0 = n * W + half * 128
                        nc.sync.dma_start(
                            out_ap[b, r0 : r0 + 128, h * D : (h + 1) * D], ao
                        )


def build_program(reps=1, split_waits=False):
    """Build the SPMD Bass program (same program on all 8 cores).

    split_waits=True applies the 1-wait-per-instruction workaround needed by
    this container's walrus build; leave False when feeding CoreSim (the
    simulator rejects the synthetic NoOp carriers).
    """
    nc = bass.Bass("TRN2", target_bir_lowering=False, debug=False)
    aps = {
        "xt": nc.dram_tensor("xt", [E, B * S], f32, kind="ExternalInput").ap(),
        "wq": nc.dram_tensor("wq", [E, C], f32, kind="ExternalInput").ap(),
        "bq": nc.dram_tensor("bq", [C], f32, kind="ExternalInput").ap(),
        "wk": nc.dram_tensor("wk", [E, C], f32, kind="ExternalInput").ap(),
        "bk": nc.dram_tensor("bk", [C], f32, kind="ExternalInput").ap(),
        "wv": nc.dram_tensor("wv", [E, C], f32, kind="ExternalInput").ap(),
        "bv": nc.dram_tensor("bv", [C], f32, kind="ExternalInput").ap(),
        "msk": nc.dram_tensor("msk", [4, 128, W], f32, kind="ExternalInput").ap(),
        "idn": nc.dram_tensor("idn", [128, 128], f32, kind="ExternalInput").ap(),
        "ones": nc.dram_tensor("ones", [1], f32, kind="ExternalInput").ap(),
        "out": nc.dram_tensor("out", [B, S, C], f32, kind="ExternalOutput").ap(),
    }
    with _TileContext(nc) as tc:
        with ExitStack() as ctx:
            sing = ctx.enter_context(tc.tile_pool(name="sing", bufs=1))
            stores = ctx.enter_context(tc.tile_pool(name="stores", bufs=1))
            for _ in range(reps):
                _emit_with_pools(nc, tc, aps, sing, stores)
    if split_waits:
        _split_sync_waits(nc)
    return nc


def _band_masks():
    yy = np.arange(128, dtype=np.int64)[:, None]
    xx = np.arange(W, dtype=np.int64)[None, :]
    m0 = np.where(yy >= xx, 0.0, MASKVAL)
    m1 = np.where(yy >= xx - 128, 0.0, MASKVAL)
    m4 = np.where(yy <= xx, 0.0, MASKVAL)
    m5 = np.where(yy <= xx - 128, 0.0, MASKVAL)
    return np.stack([m0, m1, m4, m5]).astype(np.float32)


def make_in_maps(hidden_states, Wq, bq, Wk, bk, Wv, bv):
    hs = np.ascontiguousarray(np.asarray(hidden_states, dtype=np.float32))
    xt = np.ascontiguousarray(hs.reshape(B * S, E).T)
    Wq = np.asarray(Wq, dtype=np.float32)
    Wk = np.asarray(Wk, dtype=np.float32)
    Wv = np.asarray(Wv, dtype=np.float32)
    bq = np.asarray(bq, dtype=np.float32)
    bk = np.asarray(bk, dtype=np.float32)
    bv = np.asarray(bv, dtype=np.float32)
    msk = _band_masks()
    idn = np.eye(128, dtype=np.float32)
    in_maps = []
    for r in range(NCORE):
        sl = slice(r * C, (r + 1) * C)
        in_maps.append(
            {
                "xt": xt,
                "wq": np.ascontiguousarray(Wq[:, sl]),
                "bq": np.ascontiguousarray(bq[sl]),
                "wk": np.ascontiguousarray(Wk[:, sl]),
                "bk": np.ascontiguousarray(bk[sl]),
                "wv": np.ascontiguousarray(Wv[:, sl]),
                "bv": np.ascontiguousarray(bv[sl]),
                "msk": msk,
                "idn": idn,
                "ones": np.ones([1], dtype=np.float32),
            }
        )
    return in_maps


_NC_CACHE = {}


def kernel(hidden_states, Wq, bq, Wk, bk, Wv, bv):
    if "nc" not in _NC_CACHE:
        _NC_CACHE["nc"] = build_program(split_waits=True)
    nc = _NC_CACHE["nc"]
    in_maps = make_in_maps(hidden_states, Wq, bq, Wk, bk, Wv, bv)
    res = run_bass_kernel_spmd(nc, in_maps, core_ids=list(range(NCORE)))
    out = np.concatenate(
        [res.results[r]["out"] for r in range(NCORE)], axis=2
    ).astype(np.float32)
    return out
